# revision 25
# baseline (speedup 1.0000x reference)
"""MoE all-to-all token dispatch kernel for 8 Trainium2 NeuronCores.

Problem: out[d, t*K+k, :] = x[t, :] if expert_mapping[expert_indices[t, k]] == d
else 0, with B=4, S=4096, H=512, K=2, 64 experts, 8 devices.

Strategy: the output's leading device axis is sharded across the 8 cores —
core d produces out[d] = [T*K, H].  Only ~1/8 of each core's output rows are
nonzero (each (t, k) slot is owned by exactly one device), so instead of
writing the dense 64 MiB slab, each core gathers just its owned token rows
from HBM into SBUF (dma_gather) and scatter-adds them into the owned slots of
the output (dma_scatter_add).  The output DRAM buffer is pre-zeroed by the
runtime (run_bass_kernel_spmd zero-fills/donates ExternalOutput buffers), so
untouched rows are already correct.

Routing metadata (which rows each core owns) is computed on the host from
expert_indices/expert_mapping and passed per-core as int16 index tensors.
Per-core counts are padded to a common multiple-of-CH maxn with all-valid
indices: padded gather slots read a zero row appended to xin (index T), and
padded scatter slots add those zeros to out row 0 — a no-op.  This keeps the
instruction stream fully static (one NEFF for all 8 cores, no runtime count
registers).

Work is pipelined chunk by chunk: gathers run on SWDGE queues 0/2,
scatter-adds on queues 1/3, so the SDMA engines interleave both streams.
"""

import numpy as np

B, S, H, K = 4, 4096, 512, 2
T = B * S          # 16384 tokens
TK = T * K         # 32768 output rows per device
D = 8              # devices / NeuronCores
E = 64             # experts

ZPAD = 128         # appended all-zero rows in xin (pad-slot gather targets)
ZROW = T           # index of the first zero row
CH = 512           # slots per chunk (multiple of 128)

TRACE = False
LAST_EXEC_NS = None
LAST_RESULTS = None

_CACHE = {}


def _wrap_idxs16(vals: np.ndarray, maxn: int, pad: int) -> np.ndarray:
    """SWDGE wrapped int16 layout: element i at [i % 16, i // 16], `pad`
    tail, replicated across the 8 partition groups (128 partitions)."""
    arr = np.full(maxn, pad, np.int16)
    arr[: len(vals)] = vals.astype(np.int16)
    w = arr.reshape(maxn // 16, 16).T          # [16, maxn/16]
    return np.ascontiguousarray(np.tile(w, (8, 1)))  # [128, maxn/16]


def _build_module(maxn: int):
    from contextlib import ExitStack

    import concourse.bacc as bacc
    import concourse.mybir as mybir
    from concourse.library_config import mlp

    assert maxn % CH == 0
    nb = maxn // 128
    nch = maxn // CH
    nbc = CH // 128        # data columns per chunk
    wc = CH // 16          # wrapped-idx columns per chunk

    nc = bacc.Bacc("TRN2", debug=False, num_swdge_queues=4)
    xin = nc.dram_tensor("xin", [T + ZPAD, H], mybir.dt.float32,
                         kind="ExternalInput")
    sidx = nc.dram_tensor("sidx", [128, maxn // 16], mybir.dt.int16,
                          kind="ExternalInput")
    didx = nc.dram_tensor("didx", [128, maxn // 16], mybir.dt.int16,
                          kind="ExternalInput")
    out = nc.dram_tensor("out", [TK, H], mybir.dt.float32,
                         kind="ExternalOutput")

    with (
        nc.Block() as block,
        nc.sbuf_tensor("data", [128, nb, H], mybir.dt.float32) as data,
        nc.sbuf_tensor("sidx_sb", [128, maxn // 16], mybir.dt.int16) as sidx_sb,
        nc.sbuf_tensor("didx_sb", [128, maxn // 16], mybir.dt.int16) as didx_sb,
        nc.semaphore("io0") as io0,
        nc.semaphore("io1") as io1,
        nc.semaphore("ssem0") as ssem0,
        nc.semaphore("ssem1") as ssem1,
        ExitStack() as stack,
    ):
        gsems = [stack.enter_context(nc.semaphore(f"g{c}"))  # noqa: ANT232
                 for c in range(nch)]
        LOOKAHEAD = 4

        @block.gpsimd
        def _(gpsimd):
            gpsimd.load_library(mlp)
            gpsimd.dma_start(sidx_sb[:], sidx[:]).then_inc(io0, 16)
            gpsimd.dma_start(didx_sb[:], didx[:]).then_inc(io1, 16)

            def gather(c):
                gpsimd.dma_gather(
                    data[:, c * nbc:(c + 1) * nbc, :], xin[:],
                    sidx_sb[:, c * wc:(c + 1) * wc], CH, CH, H,
                    single_packet=False, queue_num=(c % 2) * 2,
                ).then_inc(gsems[c], 16)

            # Interleave issue so scatter DGE starts as soon as its chunk's
            # gather lands instead of after every gather has been emitted.
            gpsimd.wait_ge(io0, 16)
            for c in range(min(LOOKAHEAD, nch)):
                gather(c)
            gpsimd.wait_ge(io1, 16)
            for c in range(nch):
                gpsimd.wait_ge(gsems[c], 16)
                gpsimd.dma_scatter_add(
                    out[:], data[:, c * nbc:(c + 1) * nbc, :],
                    didx_sb[:, c * wc:(c + 1) * wc], CH, CH, H,
                    single_packet=False, queue_num=(c % 2) * 2 + 1,
                ).then_inc(ssem0 if c % 2 == 0 else ssem1, 16)
                if c + LOOKAHEAD < nch:
                    gather(c + LOOKAHEAD)
            gpsimd.wait_ge(ssem0, 16 * ((nch + 1) // 2))
            gpsimd.wait_ge(ssem1, 16 * (nch // 2))

    nc.compile()
    return nc


def kernel(input_tensor, expert_indices, expert_mapping):
    global LAST_EXEC_NS, LAST_RESULTS
    from concourse.bass_utils import run_bass_kernel_spmd

    x = np.zeros((T + ZPAD, H), dtype=np.float32)
    x[:T] = np.asarray(input_tensor, dtype=np.float32).reshape(T, H)
    idx = np.asarray(expert_indices, dtype=np.int32).reshape(-1)
    emap = np.asarray(expert_mapping, dtype=np.int32)
    owner = emap[idx]                                  # [T*K], slot r = t*K+k

    dsts = [np.nonzero(owner == d)[0] for d in range(D)]
    maxn = -(-max(len(v) for v in dsts) // CH) * CH

    if maxn not in _CACHE:
        _CACHE[maxn] = _build_module(maxn)
    nc = _CACHE[maxn]

    in_maps = []
    for d in range(D):
        dst = dsts[d]
        src = dst // K
        npad = maxn - len(dst)
        # Spread pad-slot traffic: gather pads read ZPAD distinct zero rows,
        # scatter pads add those zeros to distinct UNOWNED output rows —
        # serializing thousands of RMWs on a single HBM row costs ~2x the
        # whole kernel, and padding onto owned rows races the real writes
        # (CCE read-modify-write is not atomic across engines).
        pk = np.arange(npad)
        srcfull = np.concatenate([src, ZROW + (pk % ZPAD)])
        unowned = np.ones(TK, bool)
        unowned[dst] = False
        cand = np.nonzero(unowned)[0]
        padrows = cand[:: max(1, len(cand) // max(npad, 1))][:npad]
        dstfull = np.concatenate([dst, padrows])
        in_maps.append({
            "xin": x,
            "sidx": _wrap_idxs16(srcfull, maxn, pad=0),
            "didx": _wrap_idxs16(dstfull, maxn, pad=0),
        })

    res = run_bass_kernel_spmd(nc, in_maps, list(range(D)), trace=TRACE)
    if TRACE:
        LAST_EXEC_NS = res.exec_time_ns
        LAST_RESULTS = res
    return np.stack([res.results[d]["out"] for d in range(D)], axis=0)


# revision 27
# speedup vs baseline: 1.1198x; 1.1198x over previous
"""MoE all-to-all token dispatch kernel for 8 Trainium2 NeuronCores.

Problem: out[d, t*K+k, :] = x[t, :] if expert_mapping[expert_indices[t, k]] == d
else 0, with B=4, S=4096, H=512, K=2, 64 experts, 8 devices.

Strategy: the output's leading device axis is sharded across the 8 cores —
core d produces out[d] = [T*K, H].  Only ~1/8 of each core's output rows are
nonzero (each (t, k) slot is owned by exactly one device), so instead of
writing the dense 64 MiB slab, each core gathers just its owned token rows
from HBM into SBUF (dma_gather) and scatter-adds them into the owned slots of
the output (dma_scatter_add).  The output DRAM buffer is pre-zeroed by the
runtime (run_bass_kernel_spmd zero-fills/donates ExternalOutput buffers), so
untouched rows are already correct.

Routing metadata (which rows each core owns) is computed on the host from
expert_indices/expert_mapping and passed per-core as int16 index tensors.
Per-core counts are padded to a common multiple-of-CH maxn with all-valid
indices: padded gather slots read a zero row appended to xin (index T), and
padded scatter slots add those zeros to out row 0 — a no-op.  This keeps the
instruction stream fully static (one NEFF for all 8 cores, no runtime count
registers).

Work is pipelined chunk by chunk: gathers run on SWDGE queues 0/2,
scatter-adds on queues 1/3, so the SDMA engines interleave both streams.
"""

import numpy as np

B, S, H, K = 4, 4096, 512, 2
T = B * S          # 16384 tokens
TK = T * K         # 32768 output rows per device
D = 8              # devices / NeuronCores
E = 64             # experts

ZPAD = 128         # appended all-zero rows in xin (pad-slot gather targets)
ZROW = T           # index of the first zero row
CH = 512           # slots per chunk (multiple of 128)

TRACE = False
LAST_EXEC_NS = None
LAST_RESULTS = None

_CACHE = {}


def _wrap_idxs16(vals: np.ndarray, maxn: int, pad: int) -> np.ndarray:
    """SWDGE wrapped int16 layout: element i at [i % 16, i // 16], `pad`
    tail, replicated across the 8 partition groups (128 partitions)."""
    arr = np.full(maxn, pad, np.int16)
    arr[: len(vals)] = vals.astype(np.int16)
    w = arr.reshape(maxn // 16, 16).T          # [16, maxn/16]
    return np.ascontiguousarray(np.tile(w, (8, 1)))  # [128, maxn/16]


def _build_module(maxn: int, nch_own: int):
    from contextlib import ExitStack

    import concourse.bacc as bacc
    import concourse.mybir as mybir
    from concourse.library_config import mlp

    assert maxn % CH == 0
    nb = maxn // 128
    nch = maxn // CH
    nbc = CH // 128        # data columns per chunk
    wc = CH // 16          # wrapped-idx columns per chunk

    nc = bacc.Bacc("TRN2", debug=False, num_swdge_queues=4)
    xin = nc.dram_tensor("xin", [T + ZPAD, H], mybir.dt.float32,
                         kind="ExternalInput")
    sidx = nc.dram_tensor("sidx", [128, maxn // 16], mybir.dt.int16,
                          kind="ExternalInput")
    didx = nc.dram_tensor("didx", [128, maxn // 16], mybir.dt.int16,
                          kind="ExternalInput")
    out = nc.dram_tensor("out", [TK, H], mybir.dt.float32,
                         kind="ExternalOutput")
    hlp = nc.dram_tensor("hlp", [TK, H], mybir.dt.float32,
                         kind="ExternalOutput")

    with (
        nc.Block() as block,
        nc.sbuf_tensor("data", [128, nb, H], mybir.dt.float32) as data,
        nc.sbuf_tensor("sidx_sb", [128, maxn // 16], mybir.dt.int16) as sidx_sb,
        nc.sbuf_tensor("didx_sb", [128, maxn // 16], mybir.dt.int16) as didx_sb,
        nc.semaphore("io0") as io0,
        nc.semaphore("io1") as io1,
        nc.semaphore("ssem0") as ssem0,
        nc.semaphore("ssem1") as ssem1,
        ExitStack() as stack,
    ):
        gsems = [stack.enter_context(nc.semaphore(f"g{c}"))  # noqa: ANT232
                 for c in range(nch)]
        LOOKAHEAD = 4

        @block.gpsimd
        def _(gpsimd):
            gpsimd.load_library(mlp)
            gpsimd.dma_start(sidx_sb[:], sidx[:]).then_inc(io0, 16)
            gpsimd.dma_start(didx_sb[:], didx[:]).then_inc(io1, 16)

            def gather(c):
                gpsimd.dma_gather(
                    data[:, c * nbc:(c + 1) * nbc, :], xin[:],
                    sidx_sb[:, c * wc:(c + 1) * wc], CH, CH, H,
                    single_packet=False, queue_num=(c % 2) * 2,
                ).then_inc(gsems[c], 16)

            # Interleave issue so scatter DGE starts as soon as its chunk's
            # gather lands instead of after every gather has been emitted.
            gpsimd.wait_ge(io0, 16)
            for c in range(min(LOOKAHEAD, nch)):
                gather(c)
            gpsimd.wait_ge(io1, 16)
            for c in range(nch):
                gpsimd.wait_ge(gsems[c], 16)
                tgt = out if c < nch_own else hlp
                gpsimd.dma_scatter_add(
                    tgt[:], data[:, c * nbc:(c + 1) * nbc, :],
                    didx_sb[:, c * wc:(c + 1) * wc], CH, CH, H,
                    single_packet=False, queue_num=(c % 2) * 2 + 1,
                ).then_inc(ssem0 if c % 2 == 0 else ssem1, 16)
                if c + LOOKAHEAD < nch:
                    gather(c + LOOKAHEAD)
            gpsimd.wait_ge(ssem0, 16 * ((nch + 1) // 2))
            gpsimd.wait_ge(ssem1, 16 * (nch // 2))

    nc.compile()
    return nc


def kernel(input_tensor, expert_indices, expert_mapping):
    global LAST_EXEC_NS, LAST_RESULTS
    from concourse.bass_utils import run_bass_kernel_spmd

    x = np.zeros((T + ZPAD, H), dtype=np.float32)
    x[:T] = np.asarray(input_tensor, dtype=np.float32).reshape(T, H)
    idx = np.asarray(expert_indices, dtype=np.int32).reshape(-1)
    emap = np.asarray(expert_mapping, dtype=np.int32)
    owner = emap[idx]                                  # [T*K], slot r = t*K+k

    dsts = [np.nonzero(owner == d)[0] for d in range(D)]

    # Balance: heavy slabs export 512-row tail chunks into other cores'
    # spare hlp chunk, shrinking the common per-core slot count.
    nch_own = -(-max(len(v) for v in dsts) // CH)
    for cand in range(-(-(TK // D) // CH), nch_own + 1):
        if sum(-(-max(0, len(v) - cand * CH) // CH) for v in dsts) <= D:
            nch_own = cand
            break
    exports = []                       # (src_core, rows)
    kept = []
    for d in range(D):
        n_exp = -(-max(0, len(dsts[d]) - nch_own * CH) // CH)
        kept.append(dsts[d][: len(dsts[d]) - n_exp * CH])
        for e in range(n_exp):
            lo = len(dsts[d]) - (n_exp - e) * CH
            exports.append((d, dsts[d][lo: lo + CH]))
    nch_hlp = 1 if exports else 0
    nch = nch_own + nch_hlp
    maxn = nch * CH
    imap = {}                          # importer core -> (src_core, rows)
    free = sorted(range(D), key=lambda d: len(kept[d]))
    for (sc, rows), imp in zip(exports, free):
        imap[imp] = (sc, rows)

    key = (maxn, nch_own)
    if key not in _CACHE:
        _CACHE[key] = _build_module(maxn, nch_own)
    nc = _CACHE[key]

    in_maps = []
    for d in range(D):
        own = kept[d]
        imp_rows = imap.get(d, (0, np.empty(0, np.int64)))[1]
        own_pad = nch_own * CH - len(own)
        hlp_pad = nch_hlp * CH - len(imp_rows)
        pk = np.arange(max(own_pad, hlp_pad))
        srcfull = np.concatenate([
            own // K, ZROW + (pk[:own_pad] % ZPAD),
            imp_rows // K, ZROW + (pk[:hlp_pad] % ZPAD)])
        unowned = np.ones(TK, bool)
        unowned[own] = False
        cand = np.nonzero(unowned)[0]
        own_padrows = cand[:: max(1, len(cand) // max(own_pad, 1))][:own_pad]
        unhlp = np.ones(TK, bool)
        unhlp[imp_rows] = False
        candh = np.nonzero(unhlp)[0]
        hlp_padrows = candh[:: max(1, len(candh) // max(hlp_pad, 1))][:hlp_pad]
        dstfull = np.concatenate([own, own_padrows, imp_rows, hlp_padrows])
        in_maps.append({
            "xin": x,
            "sidx": _wrap_idxs16(srcfull, maxn, pad=0),
            "didx": _wrap_idxs16(dstfull, maxn, pad=0),
        })

    res = run_bass_kernel_spmd(nc, in_maps, list(range(D)), trace=TRACE)
    if TRACE:
        LAST_EXEC_NS = res.exec_time_ns
        LAST_RESULTS = res
    outs = [np.array(res.results[d]["out"]) for d in range(D)]
    for imp, (sc, rows) in imap.items():
        outs[sc][rows] = res.results[imp]["hlp"][rows]
    return np.stack(outs, axis=0)


# revision 29
# speedup vs baseline: 1.1628x; 1.0384x over previous
"""MoE all-to-all token dispatch kernel for 8 Trainium2 NeuronCores.

Problem: out[d, t*K+k, :] = x[t, :] if expert_mapping[expert_indices[t, k]] == d
else 0, with B=4, S=4096, H=512, K=2, 64 experts, 8 devices.

Strategy: the output's leading device axis is sharded across the 8 cores —
core d produces out[d] = [T*K, H].  Only ~1/8 of each core's output rows are
nonzero (each (t, k) slot is owned by exactly one device), so instead of
writing the dense 64 MiB slab, each core gathers just its owned token rows
from HBM into SBUF (dma_gather) and scatter-adds them into the owned slots of
the output (dma_scatter_add).  The output DRAM buffer is pre-zeroed by the
runtime (run_bass_kernel_spmd zero-fills/donates ExternalOutput buffers), so
untouched rows are already correct.

Routing metadata (which rows each core owns) is computed on the host from
expert_indices/expert_mapping and passed per-core as int16 index tensors.
Per-core counts are padded to a common multiple-of-CH maxn with all-valid
indices: padded gather slots read a zero row appended to xin (index T), and
padded scatter slots add those zeros to out row 0 — a no-op.  This keeps the
instruction stream fully static (one NEFF for all 8 cores, no runtime count
registers).

Work is pipelined chunk by chunk: gathers run on SWDGE queues 0/2,
scatter-adds on queues 1/3, so the SDMA engines interleave both streams.
"""

import numpy as np

B, S, H, K = 4, 4096, 512, 2
T = B * S          # 16384 tokens
TK = T * K         # 32768 output rows per device
D = 8              # devices / NeuronCores
E = 64             # experts

ZPAD = 128         # appended all-zero rows in xin (pad-slot gather targets)
ZROW = T           # index of the first zero row
CH = 512           # slots per chunk (multiple of 128)

TRACE = False
LAST_EXEC_NS = None
LAST_RESULTS = None

_CACHE = {}


def _wrap_idxs16(vals: np.ndarray, maxn: int, pad: int) -> np.ndarray:
    """SWDGE wrapped int16 layout: element i at [i % 16, i // 16], `pad`
    tail, replicated across the 8 partition groups (128 partitions)."""
    arr = np.full(maxn, pad, np.int16)
    arr[: len(vals)] = vals.astype(np.int16)
    w = arr.reshape(maxn // 16, 16).T          # [16, maxn/16]
    return np.ascontiguousarray(np.tile(w, (8, 1)))  # [128, maxn/16]


def _build_module(maxn: int, nch_own: int | None = None):
    from contextlib import ExitStack

    import concourse.bacc as bacc
    import concourse.mybir as mybir
    from concourse.library_config import mlp

    assert maxn % CH == 0
    nb = maxn // 128
    nch = maxn // CH
    if nch_own is None:
        nch_own = nch
    nbc = CH // 128        # data columns per chunk
    wc = CH // 16          # wrapped-idx columns per chunk

    nc = bacc.Bacc("TRN2", debug=False, num_swdge_queues=4)
    xin = nc.dram_tensor("xin", [T + ZPAD, H], mybir.dt.float32,
                         kind="ExternalInput")
    sidx = nc.dram_tensor("sidx", [128, maxn // 16], mybir.dt.int16,
                          kind="ExternalInput")
    didx = nc.dram_tensor("didx", [128, maxn // 16], mybir.dt.int16,
                          kind="ExternalInput")
    out = nc.dram_tensor("out", [TK, H], mybir.dt.float32,
                         kind="ExternalOutput")
    hlp = nc.dram_tensor("hlp", [TK, H], mybir.dt.float32,
                         kind="ExternalOutput")

    with (
        nc.Block() as block,
        nc.sbuf_tensor("data", [128, nb, H], mybir.dt.float32) as data,
        nc.sbuf_tensor("sidx_sb", [128, maxn // 16], mybir.dt.int16) as sidx_sb,
        nc.sbuf_tensor("didx_sb", [128, maxn // 16], mybir.dt.int16) as didx_sb,
        nc.semaphore("io0") as io0,
        nc.semaphore("io1") as io1,
        nc.semaphore("ssem0") as ssem0,
        nc.semaphore("ssem1") as ssem1,
        nc.semaphore("ssem2") as ssem2,
        ExitStack() as stack,
    ):
        gsems = [stack.enter_context(nc.semaphore(f"g{c}"))  # noqa: ANT232
                 for c in range(nch)]
        LOOKAHEAD = 4

        @block.gpsimd
        def _(gpsimd):
            gpsimd.load_library(mlp)
            gpsimd.dma_start(sidx_sb[:], sidx[:]).then_inc(io0, 16)
            gpsimd.dma_start(didx_sb[:], didx[:]).then_inc(io1, 16)

            def gather(c):
                gpsimd.dma_gather(
                    data[:, c * nbc:(c + 1) * nbc, :], xin[:],
                    sidx_sb[:, c * wc:(c + 1) * wc], CH, CH, H,
                    single_packet=False, queue_num=0,
                ).then_inc(gsems[c], 16)

            # Interleave issue so scatter DGE starts as soon as its chunk's
            # gather lands instead of after every gather has been emitted.
            gpsimd.wait_ge(io0, 16)
            for c in range(min(LOOKAHEAD, nch)):
                gather(c)
            gpsimd.wait_ge(io1, 16)
            ssems = (ssem0, ssem1, ssem2)
            # scatters carry ~2/3 of the engine work: give them 3 of the 4
            # SWDGE rings so per-engine round-robin matches the load
            for c in range(nch):
                gpsimd.wait_ge(gsems[c], 16)
                tgt = out if c < nch_own else hlp
                gpsimd.dma_scatter_add(
                    tgt[:], data[:, c * nbc:(c + 1) * nbc, :],
                    didx_sb[:, c * wc:(c + 1) * wc], CH, CH, H,
                    single_packet=False, queue_num=1 + c % 3,
                ).then_inc(ssems[c % 3], 16)
                if c + LOOKAHEAD < nch:
                    gather(c + LOOKAHEAD)
            for q in range(3):
                gpsimd.wait_ge(ssems[q], 16 * ((nch - q + 2) // 3))

    nc.compile()
    return nc


def kernel(input_tensor, expert_indices, expert_mapping):
    global LAST_EXEC_NS, LAST_RESULTS
    from concourse.bass_utils import run_bass_kernel_spmd

    x = np.zeros((T + ZPAD, H), dtype=np.float32)
    x[:T] = np.asarray(input_tensor, dtype=np.float32).reshape(T, H)
    idx = np.asarray(expert_indices, dtype=np.int32).reshape(-1)
    emap = np.asarray(expert_mapping, dtype=np.int32)
    owner = emap[idx]                                  # [T*K], slot r = t*K+k

    dsts = [np.nonzero(owner == d)[0] for d in range(D)]

    # Balance: heavy slabs export 512-row tail chunks into other cores'
    # spare hlp chunk, shrinking the common per-core slot count.
    nch_own = -(-max(len(v) for v in dsts) // CH)
    for cand in range(-(-(TK // D) // CH), nch_own + 1):
        if sum(-(-max(0, len(v) - cand * CH) // CH) for v in dsts) <= D:
            nch_own = cand
            break
    exports = []                       # (src_core, rows)
    kept = []
    for d in range(D):
        n_exp = -(-max(0, len(dsts[d]) - nch_own * CH) // CH)
        kept.append(dsts[d][: len(dsts[d]) - n_exp * CH])
        for e in range(n_exp):
            lo = len(dsts[d]) - (n_exp - e) * CH
            exports.append((d, dsts[d][lo: lo + CH]))
    nch_hlp = 1 if exports else 0
    nch = nch_own + nch_hlp
    maxn = nch * CH
    imap = {}                          # importer core -> (src_core, rows)
    free = sorted(range(D), key=lambda d: len(kept[d]))
    for (sc, rows), imp in zip(exports, free):
        imap[imp] = (sc, rows)

    key = (maxn, nch_own)
    if key not in _CACHE:
        _CACHE[key] = _build_module(maxn, nch_own)
    nc = _CACHE[key]

    in_maps = []
    for d in range(D):
        own = kept[d]
        imp_rows = imap.get(d, (0, np.empty(0, np.int64)))[1]
        own_pad = nch_own * CH - len(own)
        hlp_pad = nch_hlp * CH - len(imp_rows)
        pk = np.arange(max(own_pad, hlp_pad))
        srcfull = np.concatenate([
            own // K, ZROW + (pk[:own_pad] % ZPAD),
            imp_rows // K, ZROW + (pk[:hlp_pad] % ZPAD)])
        unowned = np.ones(TK, bool)
        unowned[own] = False
        cand = np.nonzero(unowned)[0]
        own_padrows = cand[:: max(1, len(cand) // max(own_pad, 1))][:own_pad]
        unhlp = np.ones(TK, bool)
        unhlp[imp_rows] = False
        candh = np.nonzero(unhlp)[0]
        hlp_padrows = candh[:: max(1, len(candh) // max(hlp_pad, 1))][:hlp_pad]
        dstfull = np.concatenate([own, own_padrows, imp_rows, hlp_padrows])
        in_maps.append({
            "xin": x,
            "sidx": _wrap_idxs16(srcfull, maxn, pad=0),
            "didx": _wrap_idxs16(dstfull, maxn, pad=0),
        })

    res = run_bass_kernel_spmd(nc, in_maps, list(range(D)), trace=TRACE)
    if TRACE:
        LAST_EXEC_NS = res.exec_time_ns
        LAST_RESULTS = res
    outs = [np.array(res.results[d]["out"]) for d in range(D)]
    for imp, (sc, rows) in imap.items():
        outs[sc][rows] = res.results[imp]["hlp"][rows]
    return np.stack(outs, axis=0)


# revision 32
# speedup vs baseline: 1.2454x; 1.0711x over previous
"""MoE all-to-all token dispatch kernel for 8 Trainium2 NeuronCores.

Problem: out[d, t*K+k, :] = x[t, :] if expert_mapping[expert_indices[t, k]] == d
else 0, with B=4, S=4096, H=512, K=2, 64 experts, 8 devices.

Strategy: the output's leading device axis is sharded across the 8 cores —
core d produces out[d] = [T*K, H].  Only ~1/8 of each core's output rows are
nonzero (each (t, k) slot is owned by exactly one device), so instead of
writing the dense 64 MiB slab, each core gathers just its owned token rows
from HBM into SBUF (dma_gather) and scatter-adds them into the owned slots of
the output (dma_scatter_add).  The output DRAM buffer is pre-zeroed by the
runtime (run_bass_kernel_spmd zero-fills/donates ExternalOutput buffers), so
untouched rows are already correct.

Routing metadata (which rows each core owns) is computed on the host from
expert_indices/expert_mapping and passed per-core as int16 index tensors.
Per-core counts are padded to a common multiple-of-CH slot count with
all-valid indices: padded gather slots read one of ZPAD zero rows appended to
xin, and padded scatter slots add those zeros to distinct UNOWNED output rows
(same-row pads serialize HBM read-modify-writes and cost ~2x the kernel;
owned-row pads race the real writes).  The instruction stream is fully static
(one NEFF for all 8 cores, no runtime count registers).

Slab sizes vary ~2.7x with this routing, so heavy slabs export 512-row tail
chunks into other cores' spare "hlp" chunk (a second output tensor) to
equalize per-core slot counts; the host stitches exported rows back during
final assembly.

Work is pipelined chunk by chunk with interleaved issue: gathers on SWDGE
queue 0, scatter-adds on queues 1-3 (they carry ~2/3 of the engine work, so
they get 3 of the 4 rings in the per-engine round-robin).
"""

import numpy as np

B, S, H, K = 4, 4096, 512, 2
T = B * S          # 16384 tokens
TK = T * K         # 32768 output rows per device
D = 8              # devices / NeuronCores
E = 64             # experts

ZPAD = 128         # appended all-zero rows in xin (pad-slot gather targets)
ZROW = T           # index of the first zero row
CH = 512           # slots per chunk (multiple of 128)

TRACE = False
LAST_EXEC_NS = None
LAST_RESULTS = None

_CACHE = {}


def _wrap_idxs16(vals: np.ndarray, maxn: int, pad: int) -> np.ndarray:
    """SWDGE wrapped int16 layout: element i at [i % 16, i // 16], `pad`
    tail, replicated across the 8 partition groups (128 partitions)."""
    arr = np.full(maxn, pad, np.int16)
    arr[: len(vals)] = vals.astype(np.int16)
    w = arr.reshape(maxn // 16, 16).T          # [16, maxn/16]
    return np.ascontiguousarray(np.tile(w, (8, 1)))  # [128, maxn/16]


def _build_module(maxn: int, nch_own: int | None = None):
    from contextlib import ExitStack

    import concourse.bacc as bacc
    import concourse.mybir as mybir
    from concourse.library_config import mlp

    assert maxn % CH == 0
    nb = maxn // 128
    nch = maxn // CH
    if nch_own is None:
        nch_own = nch
    nbc = CH // 128        # data columns per chunk
    wc = CH // 16          # wrapped-idx columns per chunk

    nc = bacc.Bacc("TRN2", debug=False, num_swdge_queues=4)
    xin = nc.dram_tensor("xin", [T + ZPAD, H], mybir.dt.float32,
                         kind="ExternalInput")
    sidx = nc.dram_tensor("sidx", [128, maxn // 16], mybir.dt.int16,
                          kind="ExternalInput")
    didx = nc.dram_tensor("didx", [128, maxn // 16], mybir.dt.int16,
                          kind="ExternalInput")
    out = nc.dram_tensor("out", [TK, H], mybir.dt.float32,
                         kind="ExternalOutput")
    hlp = nc.dram_tensor("hlp", [TK, H], mybir.dt.float32,
                         kind="ExternalOutput")
    hlp2 = nc.dram_tensor("hlp2", [TK, H], mybir.dt.float32,
                          kind="ExternalOutput")

    with (
        nc.Block() as block,
        nc.sbuf_tensor("data", [128, nb, H], mybir.dt.float32) as data,
        nc.sbuf_tensor("sidx_sb", [128, maxn // 16], mybir.dt.int16) as sidx_sb,
        nc.sbuf_tensor("didx_sb", [128, maxn // 16], mybir.dt.int16) as didx_sb,
        nc.semaphore("io0") as io0,
        nc.semaphore("io1") as io1,
        nc.semaphore("ssem0") as ssem0,
        nc.semaphore("ssem1") as ssem1,
        nc.semaphore("ssem2") as ssem2,
        ExitStack() as stack,
    ):
        gsems = [stack.enter_context(nc.semaphore(f"g{c}"))  # noqa: ANT232
                 for c in range(nch)]
        LOOKAHEAD = 4

        @block.gpsimd
        def _(gpsimd):
            gpsimd.load_library(mlp)
            gpsimd.dma_start(sidx_sb[:], sidx[:]).then_inc(io0, 16)
            gpsimd.dma_start(didx_sb[:], didx[:]).then_inc(io1, 16)

            def gather(c):
                gpsimd.dma_gather(
                    data[:, c * nbc:(c + 1) * nbc, :], xin[:],
                    sidx_sb[:, c * wc:(c + 1) * wc], CH, CH, H,
                    single_packet=False, queue_num=0,
                ).then_inc(gsems[c], 16)

            # Interleave issue so scatter DGE starts as soon as its chunk's
            # gather lands instead of after every gather has been emitted.
            gpsimd.wait_ge(io0, 16)
            for c in range(min(LOOKAHEAD, nch)):
                gather(c)
            gpsimd.wait_ge(io1, 16)
            ssems = (ssem0, ssem1, ssem2)
            # scatters carry ~2/3 of the engine work: give them 3 of the 4
            # SWDGE rings so per-engine round-robin matches the load
            for c in range(nch):
                gpsimd.wait_ge(gsems[c], 16)
                tgt = out if c < nch_own else (
                    hlp if c == nch_own else hlp2)
                gpsimd.dma_scatter_add(
                    tgt[:], data[:, c * nbc:(c + 1) * nbc, :],
                    didx_sb[:, c * wc:(c + 1) * wc], CH, CH, H,
                    single_packet=False, queue_num=1 + c % 3,
                ).then_inc(ssems[c % 3], 16)
                if c + LOOKAHEAD < nch:
                    gather(c + LOOKAHEAD)
            for q in range(3):
                gpsimd.wait_ge(ssems[q], 16 * ((nch - q + 2) // 3))

    nc.compile()
    return nc


def kernel(input_tensor, expert_indices, expert_mapping):
    global LAST_EXEC_NS, LAST_RESULTS
    from concourse.bass_utils import run_bass_kernel_spmd

    x = np.zeros((T + ZPAD, H), dtype=np.float32)
    x[:T] = np.asarray(input_tensor, dtype=np.float32).reshape(T, H)
    idx = np.asarray(expert_indices, dtype=np.int32).reshape(-1)
    emap = np.asarray(expert_mapping, dtype=np.int32)
    owner = emap[idx]                                  # [T*K], slot r = t*K+k

    dsts = [np.nonzero(owner == d)[0] for d in range(D)]

    # Balance: heavy slabs export 512-row tail chunks into other cores'
    # spare hlp chunks (up to two, in separate tensors so imports from
    # different slabs cannot collide on a row index), minimizing the common
    # per-core slot count.
    max_own = -(-max(len(v) for v in dsts) // CH)
    best = (max_own, 0)
    for cand in range(max(1, -(-(TK // D) // CH) - 2), max_own + 1):
        units = sum(-(-max(0, len(v) - cand * CH) // CH) for v in dsts)
        nh = -(-units // D)
        if nh <= 2 and cand + nh < best[0] + best[1]:
            best = (cand, nh)
    nch_own, nch_hlp = best
    exports = []                       # (src_core, rows)
    kept = []
    for d in range(D):
        n_exp = -(-max(0, len(dsts[d]) - nch_own * CH) // CH)
        kept.append(dsts[d][: len(dsts[d]) - n_exp * CH])
        for e in range(n_exp):
            lo = len(dsts[d]) - (n_exp - e) * CH
            exports.append((d, dsts[d][lo: lo + CH]))
    nch = nch_own + nch_hlp
    maxn = nch * CH
    imap = {}                          # (importer, hlp_k) -> (src_core, rows)
    slots = [(imp, k) for k in range(nch_hlp)
             for imp in sorted(range(D), key=lambda d: len(kept[d]))]
    for (sc, rows), slot in zip(exports, slots):
        imap[slot] = (sc, rows)

    key = (nch_own, nch_hlp)
    if key not in _CACHE:
        _CACHE[key] = _build_module(maxn, nch_own)
    nc = _CACHE[key]

    empty = np.empty(0, np.int64)

    def _section(rows, cap):
        npad = cap - len(rows)
        mask = np.ones(TK, bool)
        mask[rows] = False
        cand_rows = np.nonzero(mask)[0]
        padrows = cand_rows[:: max(1, len(cand_rows) // max(npad, 1))][:npad]
        pk = np.arange(npad)
        s = np.concatenate([rows // K, ZROW + (pk % ZPAD)])
        t = np.concatenate([rows, padrows])
        return s, t

    in_maps = []
    for d in range(D):
        secs = [(kept[d], nch_own * CH)]
        for k in range(nch_hlp):
            secs.append((imap.get((d, k), (0, empty))[1], CH))
        parts = [_section(r, cap) for r, cap in secs]
        srcfull = np.concatenate([p[0] for p in parts])
        dstfull = np.concatenate([p[1] for p in parts])
        in_maps.append({
            "xin": x,
            "sidx": _wrap_idxs16(srcfull, maxn, pad=0),
            "didx": _wrap_idxs16(dstfull, maxn, pad=0),
        })

    res = run_bass_kernel_spmd(nc, in_maps, list(range(D)), trace=TRACE)
    if TRACE:
        LAST_EXEC_NS = res.exec_time_ns
        LAST_RESULTS = res
    outs = [np.array(res.results[d]["out"]) for d in range(D)]
    for (imp, k), (sc, rows) in imap.items():
        outs[sc][rows] = res.results[imp]["hlp" if k == 0 else "hlp2"][rows]
    return np.stack(outs, axis=0)
